# revision 16
# baseline (speedup 1.0000x reference)
"""ALBERT attention (B=2, S=2048, D=1024, H=16, K=64) on 8 TRN2 NeuronCores.

Sharding: core c = (b, g) with b = c // 4 (batch), g = c % 4 (head group of 4
heads). Each core computes output[b, :, 4g:4g+4, :] — outputs are disjoint, so
no collectives are needed.

Host-side prep: x is shipped transposed ([D, S], bf16); keys with
attention_mask == 0 are compacted away (they contribute exactly 0), padded to
a 128 multiple. Because of the compaction, only the LAST key tile contains
masked (padding) keys, so only its exp() needs the additive-mask bias.

Per-core pipeline (ScalarE exp is the roofline: 64 ACTs x ~1.1us):
  - inputs stream in chunked DMAs (critical first: wq/wk, xt, xf f-half 0;
    wv and xf f-half 1 strictly after) so the projections pipeline behind
    HBM; a dummy-matmul warmup keeps the PE HAM clock-gate at 8/8.
  - projections, weight-stationary, bf16: QT [2-head 128, S] per pair and
    KT [128, T] per pair; V computed DIRECTLY in [t, hk] layout (xt chunks
    stationary, wv moving) so no PE transpose pass is needed.
  - attention runs per (head-PAIR, f-quarter 512): the two heads' logits
    matmuls contract 64 rows each at tile_position (0,0) / (64,0) so the PE
    runs them CONCURRENTLY into one lt [128, 1024] tile ([A | B]); a single
    exp ACT covers both; per-head contexts Cacc[65, 512] += [1|V]^T @ ET
    (row 0 = softmax denominators). PSUM: lt 2x2 + cacc 2x1 + proj 2 = 8.
  - non-critical projections are chopped into small bursts and INJECTED
    between attention iterations (program order = Tile priority), so the PE
    slack under the exp stream absorbs them instead of stalling it.
  - output ships UNNORMALIZED: out[h] = [65, S] f32 (row 0 = denom,
    rows 1..65 = C^T). Host divides, transposes, and adds bv (exact since
    probs sum to 1).
"""

import ml_dtypes
import numpy as np

import concourse.bass as bass
import concourse.tile as tile
from concourse import bacc, mybir
from concourse.bass_utils import run_bass_kernel_spmd

F32 = mybir.dt.float32
BF16 = mybir.dt.bfloat16

B, S, D, H, K = 2, 2048, 1024, 16, 64
NH = 4            # heads per core
HK = NH * K       # 256
NCORES = 8
DC = D // 128     # 8 contraction chunks
NEG = -10000.0
FQ = 512          # f-quarter width
NQ = S // FQ      # 4 f-quarters


def build_nc(t_tiles: int):
    """Per-core Bass graph. t_tiles = number of 128-row key tiles after
    host-side compaction of masked-out keys."""
    T = t_tiles * 128
    tchunks = [(c, min(512, T - c)) for c in range(0, T, 512)]

    nc = bacc.Bacc("TRN2", target_bir_lowering=False, debug=False,
                   num_devices=NCORES)

    xf_d = nc.dram_tensor("xf", [D, S], BF16, kind="ExternalInput").ap()
    xt_d = nc.dram_tensor("xt", [D, T], BF16, kind="ExternalInput").ap()
    wq_d = nc.dram_tensor("wq", [D, HK], BF16, kind="ExternalInput").ap()
    wk_d = nc.dram_tensor("wk", [D, HK], BF16, kind="ExternalInput").ap()
    wv_d = nc.dram_tensor("wv", [D, HK], BF16, kind="ExternalInput").ap()
    # bias columns: [bq0 bq1 bk0 bk1] (hk-tile halves of bq / bk)
    bias_d = nc.dram_tensor("bias", [128, 4], F32, kind="ExternalInput").ap()
    # additive key mask for the LAST key tile only
    mask_d = nc.dram_tensor("mask", [128, 1], F32, kind="ExternalInput").ap()
    # unnormalized: per head, row 0 = softmax denominators, rows 1..64 = C^T
    out_d = nc.dram_tensor("out", [NH, K + 1, S], F32,
                           kind="ExternalOutput").ap()

    with tile.TileContext(nc) as tc:
        with (
            tc.sbuf_pool(name="const", bufs=1) as const_pool,
            tc.sbuf_pool(name="persist", bufs=1) as persist_pool,
            tc.psum_pool(name="proj", bufs=2) as proj_pool,
            tc.psum_pool(name="cacc", bufs=2) as cacc_pool,
            tc.sbuf_pool(name="et", bufs=t_tiles + 2) as et_pool,
            tc.sbuf_pool(name="ct", bufs=4) as ct_pool,
        ):
            bias_sb = const_pool.tile([128, 4], F32)
            mask_sb = const_pool.tile([128, 1], F32)
            warm_sb = const_pool.tile([128, 512], BF16)

            # chunk-pair tiles so consumers wait on exactly their DMA
            xt_p = [persist_pool.tile([128, 2 * T], BF16, name=f"xt{j}")
                    for j in range(4)]
            xf0_p = [persist_pool.tile([128, 2048], BF16, name=f"xf0{j}")
                     for j in range(4)]
            xf1_p = [persist_pool.tile([128, 4096], BF16, name=f"xf1{j}")
                     for j in range(2)]

            def xt_c(d):  # [128, T] view of chunk d
                return xt_p[d // 2].rearrange(
                    "p (c s) -> p c s", s=T)[:, d % 2, :]

            def xf_c(d, fh):  # [128, 1024] view of chunk (d, fh)
                if fh == 0:
                    return xf0_p[d // 2].rearrange(
                        "p (c s) -> p c s", s=1024)[:, d % 2, :]
                return xf1_p[d // 4].rearrange(
                    "p (c s) -> p c s", s=1024)[:, d % 4, :]

            wq_sb = persist_pool.tile([128, DC * HK], BF16, name="wq")
            wk_sb = persist_pool.tile([128, DC * HK], BF16, name="wk")
            wv_sb = persist_pool.tile([128, DC * HK], BF16, name="wv")
            wq_v = wq_sb.rearrange("p (c s) -> p c s", s=HK)
            wk_v = wk_sb.rearrange("p (c s) -> p c s", s=HK)
            wv_v = wv_sb.rearrange("p (c s) -> p c s", s=HK)

            qt_sb = [persist_pool.tile([128, S], BF16, name=f"qt{i}")
                     for i in range(2)]
            kt_sb = [persist_pool.tile([128, T], BF16, name=f"kt{i}")
                     for i in range(2)]
            # V with a leading ones column per head: [1|V_h0|1|V_h1|...]
            v_sb = [persist_pool.tile([128, NH * 65], BF16, name=f"v{i}")
                    for i in range(t_tiles)]
            nc.vector.memset(warm_sb[:], 0.0)
            for i in range(t_tiles):
                nc.vector.memset(
                    v_sb[i].rearrange("p (h c) -> p h c", c=65)[:, :, 0:1],
                    1.0)

            # ---------------- input DMAs ----------------
            xt_src = xt_d.rearrange("(c p) s -> p c s", p=128)
            xf_src = xf_d.rearrange("(c p) s -> p c s", p=128)
            nc.scalar.dma_start(bias_sb[:], bias_d[:])
            nc.scalar.dma_start(mask_sb[:], mask_d[:])
            nc.scalar.dma_start(
                wq_sb.rearrange("p (c s) -> p c s", s=HK),
                wq_d.rearrange("(c p) s -> p c s", p=128))
            nc.scalar.dma_start(
                wk_sb.rearrange("p (c s) -> p c s", s=HK),
                wk_d.rearrange("(c p) s -> p c s", p=128))
            for j in range(4):
                nc.sync.dma_start(
                    xt_p[j].rearrange("p (c s) -> p c s", s=T),
                    xt_src[:, 2 * j:2 * j + 2, :])
                nc.gpsimd.dma_start(
                    xf0_p[j].rearrange("p (c s) -> p c s", s=1024),
                    xf_src[:, 2 * j:2 * j + 2, 0:1024])
            # non-critical: after everything the first quarters need
            nc.scalar.dma_start(
                wv_sb.rearrange("p (c s) -> p c s", s=HK),
                wv_d.rearrange("(c p) s -> p c s", p=128))
            for j in range(2):
                nc.gpsimd.dma_start(
                    xf1_p[j].rearrange("p (c s) -> p c s", s=1024),
                    xf_src[:, 4 * j:4 * j + 4, 1024:2048])

            # -------- background projection ops (injected later) --------
            def q_mms(hk, fh, ps, d0, d1):
                for d in range(d0, d1):
                    lhs = wq_v[:, d, 128 * hk:128 * (hk + 1)]
                    for s in range(2):
                        nc.tensor.matmul(ps[s][:], lhs,
                                         xf_c(d, fh)[:, 512 * s:512 * (s + 1)],
                                         start=(d == 0), stop=(d == DC - 1))

            def q_drain(hk, fh, ps):
                for s in range(2):
                    c0 = fh * 1024 + 512 * s
                    nc.vector.tensor_scalar_add(
                        qt_sb[hk][:, c0:c0 + 512], ps[s][:],
                        bias_sb[:, hk:hk + 1])

            def bg_q_proj(hk, fh):
                """Queue-able list of small ops for one Q projection."""
                state = {}

                def alloc():
                    state["ps"] = [proj_pool.tile([128, 512], F32, tag="pp",
                                                  name=f"qp{hk}_{fh}_{s}")
                                   for s in range(2)]

                return [
                    alloc,
                    lambda: q_mms(hk, fh, state["ps"], 0, 3),
                    lambda: q_mms(hk, fh, state["ps"], 3, 6),
                    lambda: q_mms(hk, fh, state["ps"], 6, DC),
                    lambda: q_drain(hk, fh, state["ps"]),
                ]

            def bg_k_chunk(hk, s):
                c0, w = tchunks[s]
                state = {}

                def alloc():
                    state["ps"] = proj_pool.tile([128, w], F32, tag="pp",
                                                 name=f"kp{hk}_{s}")

                def mms(d0, d1):
                    for d in range(d0, d1):
                        nc.tensor.matmul(
                            state["ps"][:],
                            wk_v[:, d, 128 * hk:128 * (hk + 1)],
                            xt_c(d)[:, c0:c0 + w],
                            start=(d == 0), stop=(d == DC - 1))

                def drain():
                    nc.vector.tensor_scalar_add(
                        kt_sb[hk][:, c0:c0 + w], state["ps"][:],
                        bias_sb[:, 2 + hk:3 + hk])

                return [alloc, lambda: mms(0, 4), lambda: mms(4, DC), drain]

            def bg_v_proj(t):
                """bv is added on the host (exact: probs sum to 1)."""
                state = {}

                def alloc():
                    state["ps"] = proj_pool.tile([128, HK], F32, tag="pp",
                                                 name=f"vp{t}")

                def mms(d0, d1):
                    for d in range(d0, d1):
                        nc.tensor.matmul(
                            state["ps"][:],
                            xt_c(d)[:, 128 * t:128 * (t + 1)],
                            wv_v[:, d, :],
                            start=(d == 0), stop=(d == DC - 1))

                def drain():
                    nc.vector.tensor_copy(
                        v_sb[t].rearrange("p (h c) -> p h c",
                                          c=65)[:, :, 1:65],
                        state["ps"].rearrange("p (h c) -> p h c",
                                              c=64)[:, :, :])

                return [alloc, lambda: mms(0, 4), lambda: mms(4, DC), drain]

            bg = []          # flat list of pending (group, op) pairs

            def bg_pop(n):
                for _ in range(min(n, len(bg))):
                    bg.pop(0)[1]()

            def bg_flush(group=None):
                """Pop everything (group=None) or until no ops of `group`
                remain (list is in deadline order, so pop from the front)."""
                while bg and (group is None or
                              any(g == group for g, _ in bg)):
                    bg.pop(0)[1]()

            def attention(hk, fq, lt_pool, inject=0):
                """Both heads of pair hk on f-quarter fq: concurrent
                row-tiled logits -> one exp ACT -> per-head contexts.
                inject = background ops to pop per t-iteration."""
                hA, hB = 2 * hk, 2 * hk + 1
                c0 = FQ * fq
                caccs = [cacc_pool.tile([K + 1, FQ], F32, tag="cacc",
                                        name=f"cacc{hk}_{fq}_{i}")
                         for i in range(2)]
                for t in range(t_tiles):
                    lt = lt_pool.tile([128, 2 * FQ], F32, tag="lt",
                                      name=f"lt{hk}_{fq}_{t}")
                    for i, zo in ((0, 0), (1, 64)):
                        nc.tensor.matmul(
                            lt[:, FQ * i:FQ * (i + 1)],
                            kt_sb[hk][zo:zo + 64, 128 * t:128 * (t + 1)],
                            qt_sb[hk][zo:zo + 64, c0:c0 + FQ],
                            start=True, stop=True)
                    et = et_pool.tile([128, 2 * FQ], BF16, tag="et",
                                      name=f"et{hk}_{fq}_{t}")
                    nc.scalar.activation(
                        et[:], lt[:], mybir.ActivationFunctionType.Exp,
                        bias=(mask_sb[:, 0:1] if t == t_tiles - 1 else 0.0),
                        scale=0.125)
                    bg_pop(inject)
                    for i, h in ((0, hA), (1, hB)):
                        nc.tensor.matmul(
                            caccs[i][:],
                            v_sb[t][:, 65 * h:65 * (h + 1)],
                            et[:, FQ * i:FQ * (i + 1)],
                            start=(t == 0), stop=(t == t_tiles - 1),
                            skip_group_check=True)
                for i, h in ((0, hA), (1, hB)):
                    ct = ct_pool.tile([K + 1, FQ], F32, tag="ct",
                                      name=f"ct{hk}_{fq}_{i}")
                    nc.vector.tensor_copy(ct[:], caccs[i][:])
                    nc.sync.dma_start(out_d[h][:, c0:c0 + FQ], ct[:])

            # ---------------- schedule ----------------
            # PE warmup: keep the HAM clock-gate hot until real work lands.
            with tc.psum_pool(name="warm", bufs=1) as warm_pool:
                wps = warm_pool.tile([128, 512], F32, tag="wp", name="warm")
                for i in range(30):
                    nc.tensor.matmul(wps[:], warm_sb[:, 0:128], warm_sb[:],
                                     start=True, stop=True)

            # critical path: Q(pair0, fh0) and K(pair0) interleaved d-wise
            # in a dedicated 4-bank pool that closes before lt opens.
            with tc.psum_pool(name="early", bufs=4) as early_pool:
                q_ps = [early_pool.tile([128, 512], F32, tag="pp",
                                        name=f"qp0_0_{s}")
                        for s in range(2)]
                k_ps = early_pool.tile([128, tchunks[0][1]], F32, tag="pp",
                                       name="kp0_0")
                for d in range(DC):
                    nc.tensor.matmul(
                        k_ps[:], wk_v[:, d, 0:128],
                        xt_c(d)[:, 0:tchunks[0][1]],
                        start=(d == 0), stop=(d == DC - 1))
                    lhs = wq_v[:, d, 0:128]
                    for s in range(2):
                        nc.tensor.matmul(
                            q_ps[s][:], lhs,
                            xf_c(d, 0)[:, 512 * s:512 * (s + 1)],
                            start=(d == 0), stop=(d == DC - 1))
                nc.scalar.add(kt_sb[0][:, 0:tchunks[0][1]], k_ps[:],
                              bias_sb[:, 2:3])
                for s in range(2):
                    nc.vector.tensor_scalar_add(
                        qt_sb[0][:, 512 * s:512 * (s + 1)], q_ps[s][:],
                        bias_sb[:, 0:1])
                if len(tchunks) > 1:
                    c0, w = tchunks[1]
                    k1_ps = early_pool.tile([128, w], F32, tag="pp",
                                            name="kp0_1")
                    for d in range(DC):
                        nc.tensor.matmul(
                            k1_ps[:], wk_v[:, d, 0:128], xt_c(d)[:, c0:c0 + w],
                            start=(d == 0), stop=(d == DC - 1))
                    nc.scalar.add(kt_sb[0][:, c0:c0 + w], k1_ps[:],
                                  bias_sb[:, 2:3])

            # background work, in deadline order:
            #  - v_proj(t) must be fully issued before any context matmul
            #    that reads v_sb[t] (Tile deps follow program order)
            #  - K(0) extra chunks (t_tiles=9 case) before (0,0) tile t>=8,
            #    Q(0,fh1) before (0,2), K(1)+Q(1,0) before (1,0),
            #    Q(1,1) before (1,2)
            for s in range(2, len(tchunks)):
                bg += [("k0x", op) for op in bg_k_chunk(0, s)]
            for t in range(1, t_tiles):
                bg += [(f"v{t}", op) for op in bg_v_proj(t)]
            bg += [("q01", op) for op in bg_q_proj(0, 1)]
            for s in range(len(tchunks)):
                bg += [("k1", op) for op in bg_k_chunk(1, s)]
            bg += [("q10", op) for op in bg_q_proj(1, 0)]
            bg += [("q11", op) for op in bg_q_proj(1, 1)]

            with tc.psum_pool(name="lt", bufs=2) as lt_pool:
                # v_proj(0) must precede the first context matmul
                for op in bg_v_proj(0):
                    op()
                # (0,0): inject 4 ops/iter so v_proj(t) (4 ops) is always
                # fully issued before the iteration whose context reads it
                attention(0, 0, lt_pool, inject=4)
                if t_tiles > 1:
                    bg_flush(f"v{t_tiles - 1}")
                attention(0, 1, lt_pool, inject=1)
                bg_flush("q01")
                attention(0, 2, lt_pool, inject=2)
                attention(0, 3, lt_pool, inject=2)
                bg_flush("k1")
                bg_flush("q10")
                attention(1, 0, lt_pool, inject=1)
                bg_flush("q11")
                attention(1, 1, lt_pool, inject=1)
                bg_flush()
                attention(1, 2, lt_pool)
                attention(1, 3, lt_pool)

    nc.compile()
    return nc


_NC_CACHE = {}


def _get_nc(t_tiles: int):
    if t_tiles not in _NC_CACHE:
        _NC_CACHE[t_tiles] = build_nc(t_tiles)
    return _NC_CACHE[t_tiles]


def kernel(from_tensor, to_tensor, attention_mask, Wq, bq, Wk, bk, Wv, bv):
    from_tensor = np.asarray(from_tensor, dtype=np.float32)
    to_tensor = np.asarray(to_tensor, dtype=np.float32)
    attention_mask = np.asarray(attention_mask)
    Wq = np.asarray(Wq, dtype=np.float32)
    Wk = np.asarray(Wk, dtype=np.float32)
    Wv = np.asarray(Wv, dtype=np.float32)
    bq = np.asarray(bq, dtype=np.float32)
    bk = np.asarray(bk, dtype=np.float32)
    bv = np.asarray(bv, dtype=np.float32)

    # compact away masked-out keys (they contribute exactly 0 to the
    # context); pad to a 128 multiple and re-mask the padding tail.
    mask_np = attention_mask.astype(np.int32)
    idxs = [np.nonzero(mask_np[b])[0] for b in range(B)]
    t_eff = max(1, max(len(ix) for ix in idxs))
    T_pad = min(S, ((t_eff + 127) // 128) * 128)
    t_tiles = T_pad // 128
    nc = _get_nc(t_tiles)

    xt_c = np.zeros((B, D, T_pad), dtype=np.float32)
    maskadd = np.full((B, T_pad), NEG, dtype=np.float32)
    for b in range(B):
        ix = idxs[b]
        xt_c[b, :, :len(ix)] = to_tensor[b].T[:, ix]
        maskadd[b, :len(ix)] = 0.0

    in_maps = []
    for c in range(NCORES):
        b, g = c // 4, c % 4
        hs = slice(NH * g, NH * (g + 1))
        wq = np.ascontiguousarray(Wq[:, hs, :].reshape(D, HK))
        wk = np.ascontiguousarray(Wk[:, hs, :].reshape(D, HK))
        wv = np.ascontiguousarray(Wv[:, hs, :].reshape(D, HK))
        bias = np.stack([
            bq[hs].reshape(HK)[:128], bq[hs].reshape(HK)[128:],
            bk[hs].reshape(HK)[:128], bk[hs].reshape(HK)[128:],
        ], axis=1)
        in_maps.append({
            "xf": np.ascontiguousarray(from_tensor[b].T
                                       .astype(ml_dtypes.bfloat16)),
            "xt": np.ascontiguousarray(xt_c[b].astype(ml_dtypes.bfloat16)),
            "wq": wq.astype(ml_dtypes.bfloat16),
            "wk": wk.astype(ml_dtypes.bfloat16),
            "wv": wv.astype(ml_dtypes.bfloat16),
            "bias": np.ascontiguousarray(bias),
            "mask": np.ascontiguousarray(
                maskadd[b][(t_tiles - 1) * 128:].reshape(128, 1)),
        })

    global _LAST_IN_MAPS, _LAST_T_TILES
    _LAST_IN_MAPS = in_maps
    _LAST_T_TILES = t_tiles
    try:
        res = run_bass_kernel_spmd(nc, in_maps, core_ids=list(range(NCORES)))
    except Exception:
        # the axon terminal occasionally reports the device unrecoverable;
        # a reset + retry clears it
        try:
            import ctypes

            lib = ctypes.CDLL("/opt/axon/libaxon_pjrt.so")
            lib.axon_reset.restype = ctypes.c_int64
            lib.axon_reset()
        except Exception:
            pass
        res = run_bass_kernel_spmd(nc, in_maps, core_ids=list(range(NCORES)))

    out = np.empty((B, S, H, K), dtype=np.float32)
    for c in range(NCORES):
        b, g = c // 4, c % 4
        o = res.results[c]["out"]          # [NH, 65, S]
        ctx = o[:, 1:, :] / o[:, 0:1, :]   # normalize by denominators
        # [NH, K, S] -> [S, NH, K], plus bv
        out[b, :, NH * g:NH * (g + 1), :] = \
            ctx.transpose(2, 0, 1) + bv[NH * g:NH * (g + 1)][None]
    return out
